# revision 1
# baseline (speedup 1.0000x reference)
"""Trainium2 Bass kernel for the neural 2D min-sum LDPC decoder problem.

Strategy (v2)
-------------
Data-parallel over the batch: B=512 codewords, 64 per NeuronCore (8 cores).
Per core, per-edge state lives in SBUF with the graph on the partition axis
and the 64-batch on the free axis (256B rows).

The Tanner graph (edge_v/edge_c) is 6-regular on checks, 3-regular on
variables, built from 3 "layers": sorting each check's edges by edge id
puts exactly one edge of every variable in slots {0,1}, {2,3}, {4,5}.
Variables are relabeled by their slot-{0,1} position, which makes the
layer-0 part of both crossings contiguous.

Per iteration:
  check phase   x_j = u_j - alpha_{t-1}*c2v_j (fused), then leave-one-out
                min + sign-product min-sum -> c2v (6 slot arrays
                [128,32,64]); slots 2..5 DMA'd contiguously to DRAM.
  crossing 1    4 dma_gathers (4096x256B) fetch, for every variable, the
                c2v of its layer-1 and layer-2 edges;
                u_var = llr + alpha_t*((c2v_l0 + g_mid) + g_hi).
  crossing 2    u_var written contiguously to DRAM; 4 dma_gathers
                redistribute it to slots 2..5 position order (layer 0 is
                contiguous by construction).
All gathers: 256B rows, 4 SWDGE queues round-robin, single_packet=False,
split in halves so the next phase starts on the first half early.
alpha/beta are baked as immediates (compiled after inputs are known).
"""

import sys

for _p in ("/opt/trn_rl_repo",):
    if _p not in sys.path:
        sys.path.insert(0, _p)

import numpy as np

import concourse.bass as bass
import concourse.bacc as bacc
import concourse.mybir as mybir
import concourse.tile as tile
from concourse.bass_utils import run_bass_kernel_spmd

N = 8192          # variable nodes
M = 4096          # check nodes
DC = 6            # check degree (slots)
DV = 3            # variable degree
E = N * DV
B = 512
T = 10
NCORES = 8
BL = B // NCORES  # 64
PB = 128
GB_ = M // PB     # 32 blocks per slot array
CHUNK_BLKS = 4
NCHUNK = GB_ // CHUNK_BLKS

F32 = mybir.dt.float32
I32 = mybir.dt.int32
I16 = mybir.dt.int16
ALU = mybir.AluOpType
ACTF = mybir.ActivationFunctionType


def _derive_graph(edge_v: np.ndarray, edge_c: np.ndarray):
    """Host-side index derivation (layered 6-regular/3-regular graph)."""
    edge_v = np.asarray(edge_v, dtype=np.int64)
    edge_c = np.asarray(edge_c, dtype=np.int64)
    assert edge_v.shape == (E,) and edge_c.shape == (E,)

    order = np.argsort(edge_c, kind="stable")
    assert (edge_c[order] == np.repeat(np.arange(M), DC)).all(), (
        "graph is not 6-regular on checks"
    )
    slot_edge = order.reshape(M, DC).T.copy()  # [DC, M] edge id at (slot j, check c)

    # per-edge position
    j_of_e = np.empty(E, dtype=np.int64)
    c_of_e = np.empty(E, dtype=np.int64)
    for j in range(DC):
        j_of_e[slot_edge[j]] = j
        c_of_e[slot_edge[j]] = np.arange(M)

    # each variable must have exactly one edge in slots {0,1}, {2,3}, {4,5}
    layer_of_e = j_of_e // 2
    ve = np.full((N, 3), -1, dtype=np.int64)
    for lay in range(3):
        sel = np.where(layer_of_e == lay)[0]
        vs = edge_v[sel]
        assert len(np.unique(vs)) == N, f"layer {lay} is not a permutation"
        ve[vs, lay] = sel
    assert (ve >= 0).all()

    # storage row helpers (p-major: row = (c%128)*32 + c//128)
    rowmaj = (c_of_e % PB) * GB_ + (c_of_e // PB)
    # c2v DRAM buffer holds slots 2..5 only
    cdrow = (j_of_e - 2) * M + rowmaj          # valid for slots 2..5
    # u/llr DRAM row of a variable = its slot-{0,1} position
    fr_of_e = j_of_e * M + rowmaj              # valid for slots 0..1
    fr_of_v = fr_of_e[ve[:, 0]]                # [N]

    # u-build gathers (dst = parity pi, list pos = check c): variable at
    # (j=pi, c) -> cdram rows of its layer-1 / layer-2 edges
    ix1 = np.empty((2, M), dtype=np.int16)
    ix2 = np.empty((2, M), dtype=np.int16)
    # crossing-2 gathers (dst slot j=2..5, list pos = c): udram row of v(j,c)
    ixu = np.empty((4, M), dtype=np.int16)
    for pi in range(2):
        e = slot_edge[pi]                      # layer-0 edge at (pi, c)
        v = edge_v[e]
        ix1[pi] = cdrow[ve[v, 1]]
        ix2[pi] = cdrow[ve[v, 2]]
    for j in range(2, DC):
        v = edge_v[slot_edge[j]]
        ixu[j - 2] = fr_of_v[v]

    # host llr/output mapping: variable id at each u/llr DRAM row
    vid_of_fr = np.empty(N, dtype=np.int64)
    vid_of_fr[fr_of_v] = np.arange(N)
    return ix1, ix2, ixu, vid_of_fr


def _wrap_idx(idx_m: np.ndarray) -> np.ndarray:
    """dma_gather index layout: list position k at [k%16, k//16],
    replicated across the 8 groups of 16 partitions."""
    w = idx_m.reshape(M // 16, 16).T
    return np.tile(w, (PB // 16, 1)).copy()


def _build_program(alpha: np.ndarray, beta: np.ndarray) -> bacc.Bacc:
    nc = bacc.Bacc(num_swdge_queues=4)

    llr_t = nc.dram_tensor("llr_t", [N, BL], F32, kind="ExternalInput").ap()
    ix1_d = nc.dram_tensor("ix1", [2, PB, M // 16], I16, kind="ExternalInput").ap()
    ix2_d = nc.dram_tensor("ix2", [2, PB, M // 16], I16, kind="ExternalInput").ap()
    ixu_d = nc.dram_tensor("ixu", [4, PB, M // 16], I16, kind="ExternalInput").ap()
    post_d = nc.dram_tensor("post", [2, PB, GB_, BL], F32, kind="ExternalOutput").ap()
    bits_d = nc.dram_tensor("bits", [2, PB, GB_, BL], I32, kind="ExternalOutput").ap()
    # c2v slots 2..5, ping-pong; u_var, ping-pong
    cdrs = [
        nc.dram_tensor("cda", [4 * M, BL], F32).ap(),
        nc.dram_tensor("cdb", [4 * M, BL], F32).ap(),
    ]
    udrs = [
        nc.dram_tensor("uda", [N, BL], F32).ap(),
        nc.dram_tensor("udb", [N, BL], F32).ap(),
    ]
    cdrv = [c.rearrange("(j p g) e -> j p g e", j=4, p=PB) for c in cdrs]
    udrv = [u.rearrange("(pi p g) e -> p pi g e", pi=2, p=PB) for u in udrs]
    bitv = bits_d.rearrange("pi p g e -> p pi g e")

    QN = [0]

    def qn():
        # one queue per DMA-sem lane-pair: Tile locks each SWDGE sem lane to
        # a single queue, and lanes are assigned round-robin per gather.
        q = (QN[0] % 8) // 2
        QN[0] += 1
        return q

    with tile.TileContext(nc) as tc:
        with (
            tc.tile_pool(name="persist", bufs=1) as pp,
            tc.tile_pool(name="gbp", bufs=2) as gbp,
            tc.tile_pool(name="bits", bufs=1) as bip,
            tc.tile_pool(name="tmp", bufs=1) as tp,
            tc.tile_pool(name="ps", bufs=1, space="PSUM") as psp,
        ):
            ix1 = [pp.tile([PB, M // 16], I16, tag=f"ix1{i}", name=f"ix1{i}") for i in range(2)]
            ix2 = [pp.tile([PB, M // 16], I16, tag=f"ix2{i}", name=f"ix2{i}") for i in range(2)]
            ixu = [pp.tile([PB, M // 16], I16, tag=f"ixu{i}", name=f"ixu{i}") for i in range(4)]
            for i in range(2):
                nc.sync.dma_start(ix1[i][:], ix1_d[i])
                nc.sync.dma_start(ix2[i][:], ix2_d[i])
            for i in range(4):
                nc.sync.dma_start(ixu[i][:], ixu_d[i])

            # llr in variable(-row) order, parity-split: [128, 2, 32, 64]
            LV = pp.tile([PB, 2, GB_, BL], F32, tag="lv", name="lv")
            nc.sync.dma_start(
                LV[:], llr_t.rearrange("(pi p g) e -> p pi g e", pi=2, p=PB)
            )
            # u in position order: slots 0/1 = u_var parities, 2..5 gathered
            U = pp.tile([PB, DC, GB_, BL], F32, tag="u", name="u")
            # c2v
            C = pp.tile([PB, DC, GB_, BL], F32, tag="c", name="c")

            # t=0: u slots 0,1 = llr (variable order); 2..5 gathered from llr_t
            nc.scalar.activation(U[:, 0:2, :, :], LV[:], ACTF.Copy)
            for h in range(2):
                for i in range(4):
                    nc.gpsimd.dma_gather(
                        U[:, 2 + i, h * 16 : (h + 1) * 16, :],
                        llr_t,
                        ixu[i][:, h * 128 : (h + 1) * 128],
                        M // 2, M // 2, BL,
                        single_packet=False, queue_num=qn(),
                    )

            def check_chunk(t, ck, beta_t, alpha_p):
                """min-sum check update for chunk ck (CHUNK_BLKS blocks),
                slot-fused instructions via strided/pair-swapped AP views."""
                b0 = ck * CHUNK_BLKS
                S1 = CHUNK_BLKS * BL
                cs = C[:, :, b0 : b0 + CHUNK_BLKS, :]
                us = U[:, :, b0 : b0 + CHUNK_BLKS, :]
                if t > 0:
                    xt = psp.tile([PB, DC, CHUNK_BLKS, BL], F32, tag="x", name="xt")
                    nc.vector.scalar_tensor_tensor(
                        xt[:], cs, -alpha_p, us, ALU.mult, ALU.add
                    )
                    xs = xt[:]
                else:
                    xs = us
                mg = tp.tile([PB, DC, CHUNK_BLKS, BL], F32, tag="m", name="mg")
                sg = tp.tile([PB, DC, CHUNK_BLKS, BL], F32, tag="s", name="sg")
                nc.scalar.activation(mg[:], xs, ACTF.Abs)
                nc.scalar.activation(sg[:], xs, ACTF.Sign)
                pp3 = tp.tile([PB, 3, CHUNK_BLKS, BL], F32, tag="p3", name="pp3")
                qq3 = tp.tile([PB, 3, CHUNK_BLKS, BL], F32, tag="q3", name="qq3")
                sp3 = tp.tile([PB, 3, CHUNK_BLKS, BL], F32, tag="sp3", name="sp3")
                bsp = tp.tile([PB, CHUNK_BLKS, BL], F32, tag="bsp", name="bsp")
                ex = psp.tile([PB, DC, CHUNK_BLKS, BL], F32, tag="x", name="ex")
                # pair mins / pair sign-products (even x odd slots, strided)
                nc.vector.tensor_tensor(pp3[:], mg[:, 0::2], mg[:, 1::2], ALU.min)
                nc.vector.tensor_tensor(sp3[:], sg[:, 0::2], sg[:, 1::2], ALU.mult)
                # leave-one-pair-out mins
                nc.vector.tensor_tensor(qq3[:, 0], pp3[:, 1], pp3[:, 2], ALU.min)
                nc.vector.tensor_tensor(qq3[:, 1], pp3[:, 0], pp3[:, 2], ALU.min)
                nc.vector.tensor_tensor(qq3[:, 2], pp3[:, 0], pp3[:, 1], ALU.min)
                # leave-one-out min: E[j] = min(M[partner(j)], Q[j//2])
                mv = mg[:]
                msw = bass.AP(
                    mv.tensor, mv.offset + S1,
                    [mv.ap[0], [2 * S1, 3], [-S1, 2], [1, S1]],
                )
                qb = (qq3[:].rearrange("p a b e -> p a (b e)")[:, :, None, :]
                      .to_broadcast([PB, 3, 2, S1]))
                nc.vector.tensor_tensor(
                    ex[:].rearrange("p (a b) c e -> p a b (c e)", a=3), msw, qb, ALU.min
                )
                # total sign product * beta
                nc.vector.tensor_tensor(bsp[:], sp3[:, 0], sp3[:, 1], ALU.mult)
                nc.vector.tensor_tensor(bsp[:], bsp[:], sp3[:, 2], ALU.mult)
                nc.vector.tensor_scalar(bsp[:], bsp[:], float(beta_t), None, ALU.mult)
                # c2v = (sign * beta*sprod) * exclmin
                bb = bsp[:, None, :, :].to_broadcast([PB, DC, CHUNK_BLKS, BL])
                nc.vector.tensor_tensor(sg[:], sg[:], bb, ALU.mult)
                nc.vector.tensor_tensor(cs, sg[:], ex[:], ALU.mult)

            for t in range(T):
                beta_t = float(beta[t])
                alpha_t = float(alpha[t])
                alpha_p = float(alpha[t - 1]) if t > 0 else 0.0
                cdt, cdvt = cdrs[t % 2], cdrv[t % 2]
                udt, udvt = udrs[t % 2], udrv[t % 2]

                # --- check phase; c2v slots 2..5 -> DRAM by halves ---
                for ck in range(NCHUNK):
                    check_chunk(t, ck, beta_t, alpha_p)
                    if ck == NCHUNK // 2 - 1:
                        for j in range(2, DC):
                            nc.sync.dma_start(
                                cdvt[j - 2][:, :16, :], C[:, j, :16, :]
                            )
                for j in range(2, DC):
                    nc.sync.dma_start(cdvt[j - 2][:, 16:, :], C[:, j, 16:, :])

                last = t == T - 1
                for h in range(4):
                    hs = slice(h * 8, (h + 1) * 8)
                    ls = slice(h * 64, (h + 1) * 64)
                    gm = gbp.tile([PB, 2, 8, BL], F32, tag="gm", name="gm")
                    gh = gbp.tile([PB, 2, 8, BL], F32, tag="gh", name="gh")
                    for pi in range(2):
                        nc.gpsimd.dma_gather(
                            gm[:, pi], cdt, ix1[pi][:, ls], M // 4, M // 4, BL,
                            single_packet=False, queue_num=qn(),
                        )
                        nc.gpsimd.dma_gather(
                            gh[:, pi], cdt, ix2[pi][:, ls], M // 4, M // 4, BL,
                            single_packet=False, queue_num=qn(),
                        )
                    up = U[:, 0:2, hs, :]
                    nc.vector.tensor_tensor(up, C[:, 0:2, hs, :], gm[:], ALU.add)
                    nc.vector.tensor_tensor(up, up, gh[:], ALU.add)
                    if not last:
                        # u = llr + alpha * s
                        nc.vector.scalar_tensor_tensor(
                            up, up, alpha_t, LV[:, :, hs, :], ALU.mult, ALU.add
                        )
                        nc.sync.dma_start(udvt[:, :, hs, :], up)
                    else:
                        # posterior = llr + s ; bits = posterior < 0
                        nc.vector.tensor_tensor(up, up, LV[:, :, hs, :], ALU.add)
                        bt = bip.tile([PB, 2, 8, BL], I32, tag="bt", name="bt")
                        nc.vector.tensor_scalar(bt[:], up, 0.0, None, ALU.is_lt)
                        for pi in range(2):
                            nc.sync.dma_start(post_d[pi][:, hs, :], U[:, pi, hs, :])
                        nc.sync.dma_start(bitv[:, :, hs, :], bt[:])

                if not last:
                    # --- crossing 2: u -> position order, slots 2..5 ---
                    for h in range(4):
                        for i in range(4):
                            nc.gpsimd.dma_gather(
                                U[:, 2 + i, h * 8 : (h + 1) * 8, :],
                                udt,
                                ixu[i][:, h * 64 : (h + 1) * 64],
                                M // 4, M // 4, BL,
                                single_packet=False, queue_num=qn(),
                            )

    nc.compile()
    return nc


def _prepare(llr, edge_v, edge_c, beta, alpha):
    ix1, ix2, ixu, vid_of_fr = _derive_graph(edge_v, edge_c)
    ix1w = np.stack([_wrap_idx(ix1[i]) for i in range(2)])
    ix2w = np.stack([_wrap_idx(ix2[i]) for i in range(2)])
    ixuw = np.stack([_wrap_idx(ixu[i]) for i in range(4)])

    llr = np.asarray(llr, dtype=np.float32)
    in_maps = []
    for k in range(NCORES):
        llr_t = np.ascontiguousarray(llr[k * BL : (k + 1) * BL, vid_of_fr].T)
        in_maps.append({"llr_t": llr_t, "ix1": ix1w, "ix2": ix2w, "ixu": ixuw})
    return in_maps, vid_of_fr


def _assemble(results, vid_of_fr):
    posterior = np.empty((B, N), dtype=np.float32)
    bits = np.empty((B, N), dtype=np.int32)
    for k in range(NCORES):
        pd = results[k]["post"].reshape(N, BL)  # row = pi*4096 + p*32 + g
        bd = results[k]["bits"].reshape(N, BL)
        posterior[k * BL : (k + 1) * BL, vid_of_fr] = pd.T
        bits[k * BL : (k + 1) * BL, vid_of_fr] = bd.T
    return bits, posterior


def _run(llr, edge_v, edge_c, beta, alpha, trace=False, tmpdir=None):
    in_maps, vid_of_fr = _prepare(llr, edge_v, edge_c, beta, alpha)
    nc = _build_program(np.asarray(alpha, np.float32), np.asarray(beta, np.float32))
    res = run_bass_kernel_spmd(
        nc, in_maps, list(range(NCORES)), trace=trace, tmpdir=tmpdir
    )
    return _assemble(res.results, vid_of_fr), res


def kernel(llr, edge_v, edge_c, beta, alpha):
    (bits, posterior), _ = _run(llr, edge_v, edge_c, beta, alpha, trace=False)
    return bits, posterior



# revision 3
# speedup vs baseline: 1.2033x; 1.2033x over previous
"""Trainium2 Bass kernel for the neural 2D min-sum LDPC decoder problem.

Strategy (v4)
-------------
Data-parallel over the batch: B=512 codewords, 64 per NeuronCore (8 cores).
Per core, per-edge state lives in SBUF with the graph on the partition axis
(check c <-> partition c%128, block c//128) and the 64-batch on the free
axis (256B rows).  Variables are relabeled by their slot-{0,1} (layer-0)
position so u / llr storage row = (parity, check-row) of the layer-0 edge.

Both per-iteration crossings avoid serializing against compute:

  crossing 1 (c2v -> per-variable sums): SBUF->SBUF dma_scatter_add in
      parity-split CCE mode (sbuf_tokens_per_rank=128): slot planes 2..5
      scatter-add into 4 independent accumulator pairs SA[j] (one pair per
      plane so the 4 chains ride 4 SWDGE queues with no WAW coupling),
      chunk-by-chunk as check compute produces c2v.  dest code =
      ((g*2+parity)<<7) | p.
  u-compute   u = llr + alpha*(SA0+SA1+SA2+SA3 + c2v_l0), llr streamed
      from DRAM; u written to udram (affine HWDGE).
  crossing 2 (u -> slot positions 2..5): destination-chunked HBM gathers
      from udram; the first chunk unblocks the next iteration's first check
      chunk while the rest drain underneath its compute.

The SWDGE descriptor drain (~3ns/desc with all 4 queues busy) is the
capacity limit: 32768 descriptors x 256B per iteration.  All 4 queues are
kept busy in both phases; alpha/beta are baked as immediates.
"""

import sys

for _p in ("/opt/trn_rl_repo",):
    if _p not in sys.path:
        sys.path.insert(0, _p)

import numpy as np

import concourse.bass as bass
import concourse.bacc as bacc
import concourse.mybir as mybir
import concourse.tile as tile
from concourse.bass_utils import run_bass_kernel_spmd

N = 8192          # variable nodes
M = 4096          # check nodes
DC = 6            # check degree (slots)
DV = 3            # variable degree
E = N * DV
B = 512
T = 10
NCORES = 8
BL = B // NCORES  # 64
PB = 128
GB_ = M // PB     # 32 blocks per slot plane
CB = 4            # blocks per compute chunk
NCK = GB_ // CB   # 8 compute chunks
SB = 8            # blocks per scatter/gather chunk
NSC = GB_ // SB   # 4 scatter/gather chunks

F32 = mybir.dt.float32
I32 = mybir.dt.int32
I16 = mybir.dt.int16
ALU = mybir.AluOpType
ACTF = mybir.ActivationFunctionType


def _derive_graph(edge_v: np.ndarray, edge_c: np.ndarray):
    """Host-side index derivation (layered 6-regular/3-regular graph)."""
    edge_v = np.asarray(edge_v, dtype=np.int64)
    edge_c = np.asarray(edge_c, dtype=np.int64)
    assert edge_v.shape == (E,) and edge_c.shape == (E,)

    order = np.argsort(edge_c, kind="stable")
    assert (edge_c[order] == np.repeat(np.arange(M), DC)).all(), (
        "graph is not 6-regular on checks"
    )
    slot_edge = order.reshape(M, DC).T.copy()  # [DC, M] edge id at (slot j, check c)

    # per-edge slot / check
    j_of_e = np.empty(E, dtype=np.int64)
    c_of_e = np.empty(E, dtype=np.int64)
    for j in range(DC):
        j_of_e[slot_edge[j]] = j
        c_of_e[slot_edge[j]] = np.arange(M)

    # each variable must have exactly one edge in slots {0,1}, {2,3}, {4,5}
    layer_of_e = j_of_e // 2
    ve = np.full((N, 3), -1, dtype=np.int64)
    for lay in range(3):
        sel = np.where(layer_of_e == lay)[0]
        vs = edge_v[sel]
        assert len(np.unique(vs)) == N, f"layer {lay} is not a permutation"
        ve[vs, lay] = sel
    assert (ve >= 0).all()

    # storage: check c <-> (p = c % 128, g = c // 128); DRAM row-major p*32+g
    p_of_c = c_of_e % PB
    g_of_c = c_of_e // PB

    # u-row of a variable = its layer-0 edge position: (parity j0, p0, g0)
    e0 = ve[:, 0]
    pi_of_v = j_of_e[e0]                      # 0 or 1
    p0_of_v = p_of_c[e0]
    g0_of_v = g_of_c[e0]
    # u/llr DRAM row (linear pi*4096 + p*32 + g)
    fr_of_v = pi_of_v * M + p0_of_v * GB_ + g0_of_v
    # SBUF scatter destination code for u-row
    code_of_v = ((g0_of_v * 2 + pi_of_v) << 7) | p0_of_v

    # crossing-1 scatter lists: plane j (2..5), list pos = check c (p-fastest
    # within block): dest u-row code of the variable at (j, c)
    ixc1 = np.empty((4, M), dtype=np.int16)
    # crossing-2 / init gather lists: plane j, list pos = c: udram row of v(j,c)
    ixu = np.empty((4, M), dtype=np.int16)
    for j in range(2, DC):
        v = edge_v[slot_edge[j]]
        ixc1[j - 2] = code_of_v[v]
        ixu[j - 2] = fr_of_v[v]

    # host llr/output mapping: variable id at each u/llr DRAM row
    vid_of_fr = np.empty(N, dtype=np.int64)
    vid_of_fr[fr_of_v] = np.arange(N)
    return ixc1, ixu, vid_of_fr


def _wrap_idx(idx_m: np.ndarray) -> np.ndarray:
    """index layout: list position k at [k%16, k//16], replicated x8."""
    w = idx_m.reshape(-1, 16).T
    return np.tile(w, (PB // 16, 1)).copy()


def _build_program(alpha: np.ndarray, beta: np.ndarray) -> bacc.Bacc:
    nc = bacc.Bacc(num_swdge_queues=4)

    llr_t = nc.dram_tensor("llr_t", [N, BL], F32, kind="ExternalInput").ap()
    ixc1_d = nc.dram_tensor("ixc1", [4, NSC, PB, (M // NSC) // 16], I16,
                            kind="ExternalInput").ap()
    ixu_d = nc.dram_tensor("ixu", [4, PB, M // 16], I16, kind="ExternalInput").ap()
    post_d = nc.dram_tensor("post", [2, PB, GB_, BL], F32, kind="ExternalOutput").ap()
    bits_d = nc.dram_tensor("bits", [2, PB, GB_, BL], I32, kind="ExternalOutput").ap()
    udrs = [
        nc.dram_tensor("uda", [N, BL], F32).ap(),
        nc.dram_tensor("udb", [N, BL], F32).ap(),
    ]
    udrv = [u.rearrange("(pi p g) e -> p pi g e", pi=2, p=PB) for u in udrs]
    llrv = llr_t.rearrange("(pi p g) e -> p pi g e", pi=2, p=PB)
    bitv = bits_d.rearrange("pi p g e -> p pi g e")
    postv = post_d.rearrange("pi p g e -> p pi g e")

    QN = [0]

    def qn():
        q = QN[0] % 4
        QN[0] += 1
        return q

    with tile.TileContext(nc) as tc:
        with (
            tc.tile_pool(name="persist", bufs=1) as pp,
            tc.tile_pool(name="lbp", bufs=2) as lbp,
            tc.tile_pool(name="tmp", bufs=1) as tp,
            tc.tile_pool(name="ut", bufs=2) as utp,
            tc.tile_pool(name="ps", bufs=1, space="PSUM") as psp,
        ):
            ixc1 = [
                [pp.tile([PB, (M // NSC) // 16], I16, tag=f"ixc{j}{s}",
                         name=f"ixc{j}{s}") for s in range(NSC)]
                for j in range(4)
            ]
            ixu = [pp.tile([PB, M // 16], I16, tag=f"ixu{i}", name=f"ixu{i}")
                   for i in range(4)]
            for j in range(4):
                for s in range(NSC):
                    nc.sync.dma_start(ixc1[j][s][:], ixc1_d[j, s])
                nc.sync.dma_start(ixu[j][:], ixu_d[j])

            # u / v2c state: 3 slot-pair tiles
            U01 = pp.tile([PB, 2, GB_, BL], F32, tag="u01", name="u01")
            U23 = pp.tile([PB, 2, GB_, BL], F32, tag="u23", name="u23")
            U45 = pp.tile([PB, 2, GB_, BL], F32, tag="u45", name="u45")
            UPAIR = [U01, U23, U45]
            # c2v (all 6 slots)
            C = pp.tile([PB, DC, GB_, BL], F32, tag="c", name="c")
            # per-plane scatter accumulators (pair = parity 0/1)
            SA = [
                [pp.tile([PB, GB_, BL], F32, tag=f"sa{j}{pi}", name=f"sa{j}{pi}")
                 for pi in range(2)]
                for j in range(4)
            ]

            # init: U01 = llr (u-row order); U23/U45 gathered from llr_t
            nc.sync.dma_start(U01[:], llrv)
            for j in range(4):
                for pi in range(2):
                    nc.vector.memset(SA[j][pi][:], 0.0)
            for h in range(NSC):
                hs = slice(h * SB, (h + 1) * SB)
                ls = slice(h * (M // NSC // 16), (h + 1) * (M // NSC // 16))
                for j in range(4):
                    nc.gpsimd.dma_gather(
                        UPAIR[1 + j // 2][:, j % 2, hs, :],
                        llr_t, ixu[j][:, ls],
                        M // NSC, M // NSC, BL,
                        single_packet=False, queue_num=qn(),
                    )

            def check_chunk(t, ck, beta_t, alpha_p):
                """min-sum check update for compute chunk ck (CB blocks)."""
                b0 = ck * CB
                S1 = CB * BL
                blk = slice(b0, b0 + CB)
                cs = C[:, :, blk, :]
                if t > 0:
                    xt = psp.tile([PB, DC, CB, BL], F32, tag="x", name="xt")
                    for q in range(3):
                        nc.vector.scalar_tensor_tensor(
                            xt[:, 2 * q : 2 * q + 2],
                            C[:, 2 * q : 2 * q + 2, blk, :],
                            -alpha_p,
                            UPAIR[q][:, :, blk, :],
                            ALU.mult, ALU.add,
                        )
                    xs = xt[:]
                else:
                    xt = psp.tile([PB, DC, CB, BL], F32, tag="x", name="xt")
                    for q in range(3):
                        nc.scalar.activation(
                            xt[:, 2 * q : 2 * q + 2], UPAIR[q][:, :, blk, :],
                            ACTF.Copy,
                        )
                    xs = xt[:]
                mg = tp.tile([PB, DC, CB, BL], F32, tag="m", name="mg")
                sg = tp.tile([PB, DC, CB, BL], F32, tag="s", name="sg")
                nc.scalar.activation(mg[:], xs, ACTF.Abs)
                nc.scalar.activation(sg[:], xs, ACTF.Sign)
                pp3 = tp.tile([PB, 3, CB, BL], F32, tag="p3", name="pp3")
                qq3 = tp.tile([PB, 3, CB, BL], F32, tag="q3", name="qq3")
                sp3 = tp.tile([PB, 3, CB, BL], F32, tag="sp3", name="sp3")
                bsp = tp.tile([PB, CB, BL], F32, tag="bsp", name="bsp")
                ex = psp.tile([PB, DC, CB, BL], F32, tag="e", name="ex")
                # pair mins / pair sign-products (even x odd slots, strided)
                nc.vector.tensor_tensor(pp3[:], mg[:, 0::2], mg[:, 1::2], ALU.min)
                nc.vector.tensor_tensor(sp3[:], sg[:, 0::2], sg[:, 1::2], ALU.mult)
                # leave-one-pair-out mins
                nc.vector.tensor_tensor(qq3[:, 0], pp3[:, 1], pp3[:, 2], ALU.min)
                nc.vector.tensor_tensor(qq3[:, 1], pp3[:, 0], pp3[:, 2], ALU.min)
                nc.vector.tensor_tensor(qq3[:, 2], pp3[:, 0], pp3[:, 1], ALU.min)
                # leave-one-out min: E[j] = min(M[partner(j)], Q[j//2])
                mv = mg[:]
                msw = bass.AP(
                    mv.tensor, mv.offset + S1,
                    [mv.ap[0], [2 * S1, 3], [-S1, 2], [1, S1]],
                )
                qb = (qq3[:].rearrange("p a b e -> p a (b e)")[:, :, None, :]
                      .to_broadcast([PB, 3, 2, S1]))
                nc.vector.tensor_tensor(
                    ex[:].rearrange("p (a b) c e -> p a b (c e)", a=3), msw, qb,
                    ALU.min,
                )
                # total sign product * beta
                nc.vector.scalar_tensor_tensor(
                    bsp[:], sp3[:, 0], float(beta_t), sp3[:, 1], ALU.mult, ALU.mult
                )
                nc.vector.tensor_tensor(bsp[:], bsp[:], sp3[:, 2], ALU.mult)
                # c2v = (sign * beta*sprod) * exclmin
                bb = bsp[:, None, :, :].to_broadcast([PB, DC, CB, BL])
                nc.vector.tensor_tensor(sg[:], sg[:], bb, ALU.mult)
                nc.vector.tensor_tensor(cs, sg[:], ex[:], ALU.mult)

            for t in range(T):
                beta_t = float(beta[t])
                alpha_t = float(alpha[t])
                alpha_p = float(alpha[t - 1]) if t > 0 else 0.0
                last = t == T - 1
                udt, udvt = udrs[t % 2], udrv[t % 2]

                # --- check phase; c2v slots 2..5 scatter into SA per chunk ---
                for ck in range(NCK):
                    check_chunk(t, ck, beta_t, alpha_p)
                    if ck % (NCK // NSC) == (NCK // NSC) - 1:
                        sck = ck // (NCK // NSC)
                        sbs = slice(sck * SB, (sck + 1) * SB)
                        for j in range(4):
                            nc.gpsimd.dma_scatter_add(
                                SA[j][0][:],
                                C[:, 2 + j, sbs, :],
                                ixc1[j][sck][:],
                                M // NSC, M // NSC, BL,
                                single_packet=True,
                                queue_num=j,
                                sbuf_tokens_per_rank=PB,
                                parity_reg=0,
                                out_ap_other=SA[j][1][:],
                            )

                # --- u-compute per (parity, 8-block) chunk; stream llr ---
                for pi in range(2):
                    for h in range(NSC):
                        hs = slice(h * SB, (h + 1) * SB)
                        lt = lbp.tile([PB, SB, BL], F32, tag="lt", name="lt")
                        nc.sync.dma_start(lt[:], llrv[:, pi, hs, :])
                        ua = utp.tile([PB, SB, BL], F32, tag="ua", name="ua")
                        ub = utp.tile([PB, SB, BL], F32, tag="ub", name="ub")
                        nc.vector.tensor_tensor(
                            ua[:], SA[0][pi][:, hs, :], SA[1][pi][:, hs, :], ALU.add
                        )
                        nc.vector.tensor_tensor(
                            ub[:], SA[2][pi][:, hs, :], SA[3][pi][:, hs, :], ALU.add
                        )
                        nc.vector.tensor_tensor(ua[:], ua[:], ub[:], ALU.add)
                        nc.vector.tensor_tensor(
                            ua[:], ua[:], C[:, pi, hs, :], ALU.add
                        )
                        up = U01[:, pi, hs, :]
                        if not last:
                            # u = llr + alpha * s
                            nc.vector.scalar_tensor_tensor(
                                up, ua[:], alpha_t, lt[:], ALU.mult, ALU.add
                            )
                            nc.sync.dma_start(udvt[:, pi, hs, :], up)
                        else:
                            # posterior = llr + s ; bits = posterior < 0
                            nc.vector.tensor_tensor(up, ua[:], lt[:], ALU.add)
                            bt = utp.tile([PB, SB, BL], I32, tag="bt", name="bt")
                            nc.vector.tensor_scalar(
                                bt[:], up, 0.0, None, ALU.is_lt
                            )
                            nc.sync.dma_start(postv[:, pi, hs, :], up)
                            nc.sync.dma_start(bitv[:, pi, hs, :], bt[:])
                        # zero the consumed SA chunks for the next iteration
                        if not last:
                            for j in range(4):
                                nc.vector.tensor_scalar(
                                    SA[j][pi][:, hs, :], SA[j][pi][:, hs, :],
                                    0.0, None, ALU.mult,
                                )

                if not last:
                    # --- crossing 2: dest-chunked gathers u -> slots 2..5 ---
                    for h in range(NSC):
                        hs = slice(h * SB, (h + 1) * SB)
                        ls = slice(h * (M // NSC // 16), (h + 1) * (M // NSC // 16))
                        for j in range(4):
                            nc.gpsimd.dma_gather(
                                UPAIR[1 + j // 2][:, j % 2, hs, :],
                                udt, ixu[j][:, ls],
                                M // NSC, M // NSC, BL,
                                single_packet=False, queue_num=qn(),
                            )

    nc.compile()
    return nc


def _prepare(llr, edge_v, edge_c, beta, alpha):
    ixc1, ixu, vid_of_fr = _derive_graph(edge_v, edge_c)
    # crossing-1 lists chunked by source blocks: chunk s covers checks with
    # g in [s*SB,(s+1)*SB) -> list pos within chunk = (g-g0)*128 + p = c - s*1024
    ixc1w = np.stack([
        np.stack([_wrap_idx(ixc1[j][s * (M // NSC):(s + 1) * (M // NSC)])
                  for s in range(NSC)])
        for j in range(4)
    ])
    ixuw = np.stack([_wrap_idx(ixu[i]) for i in range(4)])

    llr = np.asarray(llr, dtype=np.float32)
    in_maps = []
    for k in range(NCORES):
        llr_t = np.ascontiguousarray(llr[k * BL: (k + 1) * BL, vid_of_fr].T)
        in_maps.append({"llr_t": llr_t, "ixc1": ixc1w, "ixu": ixuw})
    return in_maps, vid_of_fr


def _assemble(results, vid_of_fr):
    posterior = np.empty((B, N), dtype=np.float32)
    bits = np.empty((B, N), dtype=np.int32)
    for k in range(NCORES):
        pd = results[k]["post"].reshape(N, BL)  # row = pi*4096 + p*32 + g
        bd = results[k]["bits"].reshape(N, BL)
        posterior[k * BL: (k + 1) * BL, vid_of_fr] = pd.T
        bits[k * BL: (k + 1) * BL, vid_of_fr] = bd.T
    return bits, posterior


def _run(llr, edge_v, edge_c, beta, alpha, trace=False, tmpdir=None):
    in_maps, vid_of_fr = _prepare(llr, edge_v, edge_c, beta, alpha)
    nc = _build_program(np.asarray(alpha, np.float32), np.asarray(beta, np.float32))
    res = run_bass_kernel_spmd(
        nc, in_maps, list(range(NCORES)), trace=trace, tmpdir=tmpdir
    )
    return _assemble(res.results, vid_of_fr), res


def kernel(llr, edge_v, edge_c, beta, alpha):
    (bits, posterior), _ = _run(llr, edge_v, edge_c, beta, alpha, trace=False)
    return bits, posterior


# revision 4
# speedup vs baseline: 1.3842x; 1.1503x over previous
"""Trainium2 Bass kernel for the neural 2D min-sum LDPC decoder problem.

Strategy (v5)
-------------
Data-parallel over the batch: B=512 codewords, 64 per NeuronCore (8 cores).
Per core, per-edge state lives in SBUF with the graph on the partition axis
(check c <-> partition c%128, block c//128) and the 64-batch on the free
axis (256B rows).  Variables are relabeled by their slot-{0,1} (layer-0)
position so u / llr storage row = (parity, check-row) of the layer-0 edge.

Both per-iteration crossings pipeline with compute at 4-block granularity:

  crossing 1 (c2v -> per-variable sums): SBUF->SBUF dma_scatter_add in
      parity-split CCE mode (sbuf_tokens_per_rank=128).  Slot plane 2+j
      scatter-adds into its own accumulator pair SA[j] on queue j (4
      independent WAW chains ride 4 SWDGE queues); a 512-descriptor wave
      fires after every check compute chunk, so the chains drain in
      lockstep with compute.  dest code = ((g*2+parity)<<7) | p.
  u-compute   u = llr + alpha*(SA0+SA1+SA2+SA3 + c2v_l0), llr streamed
      from DRAM; u written to udram (affine HWDGE).
  crossing 2 (u -> slot positions 2..5): destination-chunked HBM gathers
      from udram in 512-descriptor waves; wave k unblocks check chunk k of
      the next iteration while later waves drain underneath its compute.

The SWDGE descriptor drain (~3ns/desc pipelined, ~12ns/desc on a WAW
chain) is the capacity limit: 32768 descriptors x 256B per iteration.
alpha/beta are baked as immediates (compiled after inputs are known).
"""

import sys

for _p in ("/opt/trn_rl_repo",):
    if _p not in sys.path:
        sys.path.insert(0, _p)

import numpy as np

import concourse.bass as bass
import concourse.bacc as bacc
import concourse.mybir as mybir
import concourse.tile as tile
from concourse.bass_utils import run_bass_kernel_spmd

N = 8192          # variable nodes
M = 4096          # check nodes
DC = 6            # check degree (slots)
DV = 3            # variable degree
E = N * DV
B = 512
T = 10
NCORES = 8
BL = B // NCORES  # 64
PB = 128
GB_ = M // PB     # 32 blocks per slot plane
CB = 4            # blocks per compute / scatter / gather chunk
NCK = GB_ // CB   # 8 chunks
CM = M // NCK     # 512 tokens per chunk

F32 = mybir.dt.float32
I32 = mybir.dt.int32
I16 = mybir.dt.int16
ALU = mybir.AluOpType
ACTF = mybir.ActivationFunctionType


def _derive_graph(edge_v: np.ndarray, edge_c: np.ndarray):
    """Host-side index derivation (layered 6-regular/3-regular graph)."""
    edge_v = np.asarray(edge_v, dtype=np.int64)
    edge_c = np.asarray(edge_c, dtype=np.int64)
    assert edge_v.shape == (E,) and edge_c.shape == (E,)

    order = np.argsort(edge_c, kind="stable")
    assert (edge_c[order] == np.repeat(np.arange(M), DC)).all(), (
        "graph is not 6-regular on checks"
    )
    slot_edge = order.reshape(M, DC).T.copy()  # [DC, M] edge id at (slot j, check c)

    j_of_e = np.empty(E, dtype=np.int64)
    c_of_e = np.empty(E, dtype=np.int64)
    for j in range(DC):
        j_of_e[slot_edge[j]] = j
        c_of_e[slot_edge[j]] = np.arange(M)

    # each variable must have exactly one edge in slots {0,1}, {2,3}, {4,5}
    layer_of_e = j_of_e // 2
    ve = np.full((N, 3), -1, dtype=np.int64)
    for lay in range(3):
        sel = np.where(layer_of_e == lay)[0]
        vs = edge_v[sel]
        assert len(np.unique(vs)) == N, f"layer {lay} is not a permutation"
        ve[vs, lay] = sel
    assert (ve >= 0).all()

    # storage: check c <-> (p = c % 128, g = c // 128)
    p_of_c = c_of_e % PB
    g_of_c = c_of_e // PB

    # u-row of a variable = its layer-0 edge position: (parity j0, p0, g0)
    e0 = ve[:, 0]
    pi_of_v = j_of_e[e0]
    p0_of_v = p_of_c[e0]
    g0_of_v = g_of_c[e0]
    fr_of_v = pi_of_v * M + p0_of_v * GB_ + g0_of_v       # u/llr DRAM row
    code_of_v = ((g0_of_v * 2 + pi_of_v) << 7) | p0_of_v  # SBUF scatter code

    # crossing-1 scatter lists: plane j (2..5), list pos = check c
    ixc1 = np.empty((4, M), dtype=np.int16)
    # crossing-2 / init gather lists: plane j, list pos = c: udram row of v(j,c)
    ixu = np.empty((4, M), dtype=np.int16)
    for j in range(2, DC):
        v = edge_v[slot_edge[j]]
        ixc1[j - 2] = code_of_v[v]
        ixu[j - 2] = fr_of_v[v]

    vid_of_fr = np.empty(N, dtype=np.int64)
    vid_of_fr[fr_of_v] = np.arange(N)
    return ixc1, ixu, vid_of_fr


def _wrap_idx(idx_m: np.ndarray) -> np.ndarray:
    """index layout: list position k at [k%16, k//16], replicated x8."""
    w = idx_m.reshape(-1, 16).T
    return np.tile(w, (PB // 16, 1)).copy()


def _build_program(alpha: np.ndarray, beta: np.ndarray) -> bacc.Bacc:
    nc = bacc.Bacc(num_swdge_queues=4)

    llr_t = nc.dram_tensor("llr_t", [N, BL], F32, kind="ExternalInput").ap()
    ixc1_d = nc.dram_tensor("ixc1", [4, NCK, PB, CM // 16], I16,
                            kind="ExternalInput").ap()
    ixu_d = nc.dram_tensor("ixu", [4, PB, M // 16], I16, kind="ExternalInput").ap()
    post_d = nc.dram_tensor("post", [2, PB, GB_, BL], F32, kind="ExternalOutput").ap()
    bits_d = nc.dram_tensor("bits", [2, PB, GB_, BL], I32, kind="ExternalOutput").ap()
    udrs = [
        nc.dram_tensor("uda", [N, BL], F32).ap(),
        nc.dram_tensor("udb", [N, BL], F32).ap(),
    ]
    udrv = [u.rearrange("(pi p g) e -> p pi g e", pi=2, p=PB) for u in udrs]
    llrv = llr_t.rearrange("(pi p g) e -> p pi g e", pi=2, p=PB)
    bitv = bits_d.rearrange("pi p g e -> p pi g e")
    postv = post_d.rearrange("pi p g e -> p pi g e")

    QN = [0]

    def qn():
        q = QN[0] % 4
        QN[0] += 1
        return q

    with tile.TileContext(nc) as tc:
        with (
            tc.tile_pool(name="persist", bufs=1) as pp,
            tc.tile_pool(name="lbp", bufs=2) as lbp,
            tc.tile_pool(name="tmp", bufs=1) as tp,
            tc.tile_pool(name="ut", bufs=1) as utp,
            tc.tile_pool(name="ps", bufs=1, space="PSUM") as psp,
        ):
            ixc1 = [
                [pp.tile([PB, CM // 16], I16, tag=f"ixc{j}{s}",
                         name=f"ixc{j}{s}") for s in range(NCK)]
                for j in range(4)
            ]
            ixu = [pp.tile([PB, M // 16], I16, tag=f"ixu{i}", name=f"ixu{i}")
                   for i in range(4)]
            for j in range(4):
                for s in range(NCK):
                    nc.sync.dma_start(ixc1[j][s][:], ixc1_d[j, s])
                nc.sync.dma_start(ixu[j][:], ixu_d[j])

            # u (slots 0,1) / gathered u (slots 2..5); x = u - alpha*c2v
            U = pp.tile([PB, DC, GB_, BL], F32, tag="u", name="u")
            # c2v (all 6 slots)
            C = pp.tile([PB, DC, GB_, BL], F32, tag="c", name="c")
            # per-plane scatter accumulator pairs (parity 0/1)
            SA = [
                [pp.tile([PB, GB_, BL], F32, tag=f"sa{j}{pi}", name=f"sa{j}{pi}")
                 for pi in range(2)]
                for j in range(4)
            ]

            # init: U slots 0,1 = llr (u-row order); slots 2..5 gathered
            nc.sync.dma_start(
                U[:, 0:2, :, :], llrv
            )
            for j in range(4):
                for pi in range(2):
                    nc.vector.memset(SA[j][pi][:], 0.0)
            for ck in range(NCK):
                hs = slice(ck * CB, (ck + 1) * CB)
                ls = slice(ck * (CM // 16), (ck + 1) * (CM // 16))
                for j in range(4):
                    nc.gpsimd.dma_gather(
                        U[:, 2 + j, hs, :], llr_t, ixu[j][:, ls],
                        CM, CM, BL,
                        single_packet=False, queue_num=qn(),
                    )

            def check_chunk(t, ck, beta_t, alpha_p):
                """min-sum check update for compute chunk ck (CB blocks)."""
                b0 = ck * CB
                S1 = CB * BL
                blk = slice(b0, b0 + CB)
                cs = C[:, :, blk, :]
                us = U[:, :, blk, :]
                if t > 0:
                    xt = psp.tile([PB, DC, CB, BL], F32, tag="x", name="xt")
                    nc.vector.scalar_tensor_tensor(
                        xt[:], cs, -alpha_p, us, ALU.mult, ALU.add
                    )
                    xs = xt[:]
                else:
                    xs = us
                mg = tp.tile([PB, DC, CB, BL], F32, tag="m", name="mg")
                sg = tp.tile([PB, DC, CB, BL], F32, tag="s", name="sg")
                nc.scalar.activation(mg[:], xs, ACTF.Abs)
                nc.scalar.activation(sg[:], xs, ACTF.Sign)
                pp3 = tp.tile([PB, 3, CB, BL], F32, tag="p3", name="pp3")
                qq3 = tp.tile([PB, 3, CB, BL], F32, tag="q3", name="qq3")
                sp3 = tp.tile([PB, 3, CB, BL], F32, tag="sp3", name="sp3")
                bsp = tp.tile([PB, CB, BL], F32, tag="bsp", name="bsp")
                ex = psp.tile([PB, DC, CB, BL], F32, tag="e", name="ex")
                # pair mins / pair sign-products (even x odd slots, strided)
                nc.vector.tensor_tensor(pp3[:], mg[:, 0::2], mg[:, 1::2], ALU.min)
                nc.vector.tensor_tensor(sp3[:], sg[:, 0::2], sg[:, 1::2], ALU.mult)
                # leave-one-pair-out mins
                nc.vector.tensor_tensor(qq3[:, 0], pp3[:, 1], pp3[:, 2], ALU.min)
                nc.vector.tensor_tensor(qq3[:, 1], pp3[:, 0], pp3[:, 2], ALU.min)
                nc.vector.tensor_tensor(qq3[:, 2], pp3[:, 0], pp3[:, 1], ALU.min)
                # leave-one-out min: E[j] = min(M[partner(j)], Q[j//2])
                mv = mg[:]
                msw = bass.AP(
                    mv.tensor, mv.offset + S1,
                    [mv.ap[0], [2 * S1, 3], [-S1, 2], [1, S1]],
                )
                qb = (qq3[:].rearrange("p a b e -> p a (b e)")[:, :, None, :]
                      .to_broadcast([PB, 3, 2, S1]))
                nc.vector.tensor_tensor(
                    ex[:].rearrange("p (a b) c e -> p a b (c e)", a=3), msw, qb,
                    ALU.min,
                )
                # total sign product * beta
                nc.vector.scalar_tensor_tensor(
                    bsp[:], sp3[:, 0], float(beta_t), sp3[:, 1], ALU.mult, ALU.mult
                )
                nc.vector.tensor_tensor(bsp[:], bsp[:], sp3[:, 2], ALU.mult)
                # c2v = (sign * beta*sprod) * exclmin
                bb = bsp[:, None, :, :].to_broadcast([PB, DC, CB, BL])
                nc.vector.tensor_tensor(sg[:], sg[:], bb, ALU.mult)
                nc.vector.tensor_tensor(cs, sg[:], ex[:], ALU.mult)

            for t in range(T):
                beta_t = float(beta[t])
                alpha_t = float(alpha[t])
                alpha_p = float(alpha[t - 1]) if t > 0 else 0.0
                last = t == T - 1
                udt, udvt = udrs[t % 2], udrv[t % 2]

                # --- check phase; a 4-plane scatter wave after every chunk ---
                for ck in range(NCK):
                    check_chunk(t, ck, beta_t, alpha_p)
                    sbs = slice(ck * CB, (ck + 1) * CB)
                    for j in range(4):
                        nc.gpsimd.dma_scatter_add(
                            SA[j][0][:],
                            C[:, 2 + j, sbs, :],
                            ixc1[j][ck][:],
                            CM, CM, BL,
                            single_packet=True,
                            queue_num=j,
                            sbuf_tokens_per_rank=PB,
                            parity_reg=0,
                            out_ap_other=SA[j][1][:],
                        )

                # --- u-compute per (parity, 16-block) chunk; stream llr ---
                for pi in range(2):
                    for h in range(2):
                        hs = slice(h * 16, (h + 1) * 16)
                        lt = lbp.tile([PB, 16, BL], F32, tag="lt", name="lt")
                        nc.sync.dma_start(lt[:], llrv[:, pi, hs, :])
                        ua = utp.tile([PB, 16, BL], F32, tag="ua", name="ua")
                        ub = utp.tile([PB, 16, BL], F32, tag="ub", name="ub")
                        nc.vector.tensor_tensor(
                            ua[:], SA[0][pi][:, hs, :], SA[1][pi][:, hs, :], ALU.add
                        )
                        nc.vector.tensor_tensor(
                            ub[:], SA[2][pi][:, hs, :], SA[3][pi][:, hs, :], ALU.add
                        )
                        nc.vector.tensor_tensor(ua[:], ua[:], ub[:], ALU.add)
                        nc.vector.tensor_tensor(
                            ua[:], ua[:], C[:, pi, hs, :], ALU.add
                        )
                        up = U[:, pi, hs, :]
                        if not last:
                            # u = llr + alpha * s
                            nc.vector.scalar_tensor_tensor(
                                up, ua[:], alpha_t, lt[:], ALU.mult, ALU.add
                            )
                            nc.sync.dma_start(udvt[:, pi, hs, :], up)
                        else:
                            # posterior = llr + s ; bits = posterior < 0
                            nc.vector.tensor_tensor(up, ua[:], lt[:], ALU.add)
                            bt = utp.tile([PB, 16, BL], I32, tag="bt", name="bt")
                            nc.vector.tensor_scalar(
                                bt[:], up, 0.0, None, ALU.is_lt
                            )
                            nc.sync.dma_start(postv[:, pi, hs, :], up)
                            nc.sync.dma_start(bitv[:, pi, hs, :], bt[:])
                    # zero this parity's accumulators for the next iteration
                    if not last:
                        for j in range(4):
                            nc.scalar.activation(
                                SA[j][pi][:], SA[j][pi][:], ACTF.Copy, scale=0.0
                            )

                if not last:
                    # --- crossing 2: dest-chunked gather waves u -> slots ---
                    for ck in range(NCK):
                        hs = slice(ck * CB, (ck + 1) * CB)
                        ls = slice(ck * (CM // 16), (ck + 1) * (CM // 16))
                        for j in range(4):
                            nc.gpsimd.dma_gather(
                                U[:, 2 + j, hs, :], udt, ixu[j][:, ls],
                                CM, CM, BL,
                                single_packet=False, queue_num=qn(),
                            )

    nc.compile()
    return nc


def _prepare(llr, edge_v, edge_c, beta, alpha):
    ixc1, ixu, vid_of_fr = _derive_graph(edge_v, edge_c)
    # crossing-1 lists chunked by source blocks (chunk s = checks
    # [s*512,(s+1)*512) in p-fastest order)
    ixc1w = np.stack([
        np.stack([_wrap_idx(ixc1[j][s * CM:(s + 1) * CM]) for s in range(NCK)])
        for j in range(4)
    ])
    ixuw = np.stack([_wrap_idx(ixu[i]) for i in range(4)])

    llr = np.asarray(llr, dtype=np.float32)
    in_maps = []
    for k in range(NCORES):
        llr_t = np.ascontiguousarray(llr[k * BL: (k + 1) * BL, vid_of_fr].T)
        in_maps.append({"llr_t": llr_t, "ixc1": ixc1w, "ixu": ixuw})
    return in_maps, vid_of_fr


def _assemble(results, vid_of_fr):
    posterior = np.empty((B, N), dtype=np.float32)
    bits = np.empty((B, N), dtype=np.int32)
    for k in range(NCORES):
        pd = results[k]["post"].reshape(N, BL)  # row = pi*4096 + p*32 + g
        bd = results[k]["bits"].reshape(N, BL)
        posterior[k * BL: (k + 1) * BL, vid_of_fr] = pd.T
        bits[k * BL: (k + 1) * BL, vid_of_fr] = bd.T
    return bits, posterior


def _run(llr, edge_v, edge_c, beta, alpha, trace=False, tmpdir=None):
    in_maps, vid_of_fr = _prepare(llr, edge_v, edge_c, beta, alpha)
    nc = _build_program(np.asarray(alpha, np.float32), np.asarray(beta, np.float32))
    res = run_bass_kernel_spmd(
        nc, in_maps, list(range(NCORES)), trace=trace, tmpdir=tmpdir
    )
    return _assemble(res.results, vid_of_fr), res


def kernel(llr, edge_v, edge_c, beta, alpha):
    (bits, posterior), _ = _run(llr, edge_v, edge_c, beta, alpha, trace=False)
    return bits, posterior
